# revision 1
# baseline (speedup 1.0000x reference)
"""Trainium2 Bass kernel for nn_ClassifierGuided (2-modality top-12-of-16 MoE classifier).

Sharding: pure data-parallel over tokens. 2 modalities x 4096 tokens = 8192
tokens; each of the 8 cores owns 1024 tokens of one modality (cores 0-3 ->
modality 0, cores 4-7 -> modality 1) and that modality's full weights.
Dense-eval MoE (all 16 experts computed, sparse gates applied), so no
all-to-all is needed.

Per-core math (transposed activation layout, d on partitions):
  gating   : logits = x @ Wg -> top-12 mask -> softmax -> gates g [B,16]
  experts  : h_e = relu(x @ W1_e + b1_e); hg_e = g_e * h_e
  combine  : moeT = sum_e W2_e^T @ hg_e  (+ b2^T @ g^T folded as one matmul)
  residual : z = relu(moe) + x
  head     : outT = Wo^T @ zT + bo

All matmuls run as float32r (full-rate fp32 PE path, ~1e-4 relative rounding).
Experts are processed in pairs so every expert matmul is a full 128x128 tile
(192+192 = 384 = 3*128 h-dims per pair).
"""
import sys

sys.path.insert(0, "/opt/trn_rl_repo")

import numpy as np

import concourse.bass as bass
import concourse.mybir as mybir
import concourse.tile as tile
from concourse import bacc
from concourse.bass_utils import run_bass_kernel_spmd
from concourse.masks import make_identity

# ---- problem sizes (hardcoded per the harness contract) ----
B = 4096           # tokens per modality
D = 768            # model dim
E = 16             # experts
H = 192            # expert hidden
O = 101            # classifier out
KTOP = 12          # top-k experts
NCORES = 8
BC = B // 4        # 1024 tokens per core
DC = D // 128      # 6 d-chunks
NT = 512           # token tile (matmul moving dim)
NTILES = BC // NT  # 2
NPAIR = E // 2     # 8 expert pairs
HP = 2 * H         # 384 h-dims per pair = 3 chunks of 128
HCH = HP // 128    # 3
F32 = mybir.dt.float32
F32R = mybir.dt.float32r
NEG_BIG = -1.0e30

_NC_CACHE = {}
DEBUG = False


def build_nc():
    nc = bacc.Bacc("TRN2", target_bir_lowering=False, debug=False,
                   num_devices=NCORES)

    # ---- DRAM I/O (per-core views; host pre-packs) ----
    xT = nc.dram_tensor("xT", [D, BC], F32R, kind="ExternalInput").ap()
    w1p = nc.dram_tensor("w1p", [D, E * H], F32R, kind="ExternalInput").ap()
    w2p = nc.dram_tensor("w2p", [E * H, D], F32R, kind="ExternalInput").ap()
    b1p = nc.dram_tensor("b1p", [128, E * H // 128], F32, kind="ExternalInput").ap()
    b2 = nc.dram_tensor("b2", [E, D], F32R, kind="ExternalInput").ap()
    wg = nc.dram_tensor("wg", [D, E], F32R, kind="ExternalInput").ap()
    wo = nc.dram_tensor("wo", [D, O], F32R, kind="ExternalInput").ap()
    bo = nc.dram_tensor("bo", [O, 1], F32, kind="ExternalInput").ap()
    outT = nc.dram_tensor("outT", [O, BC], F32, kind="ExternalOutput").ap()
    if DEBUG:
        dbg_gT = nc.dram_tensor("dbg_gT", [E, BC], F32, kind="ExternalOutput").ap()
        dbg_gb = nc.dram_tensor("dbg_gb", [128, 2, NT], F32, kind="ExternalOutput").ap()
        dbg_hg = nc.dram_tensor("dbg_hg", [128, NT], F32, kind="ExternalOutput").ap()
        dbg_h = nc.dram_tensor("dbg_h", [128, NT], F32, kind="ExternalOutput").ap()
        dbg_moe = nc.dram_tensor("dbg_moe", [128, DC, NT], F32, kind="ExternalOutput").ap()

    with tile.TileContext(nc) as tc:
        with tc.tile_pool(name="const", bufs=1) as cpool:
            # resident SBUF tensors
            xsb = cpool.tile([128, DC, BC], F32R)          # xT; later holds zT
            w1sb = cpool.tile([128, DC, E * H], F32R)
            b1sb = cpool.tile([128, E * H // 128], F32)
            b2sb = cpool.tile([E, D], F32R)
            wosb = cpool.tile([128, DC, O], F32R)
            bosb = cpool.tile([O, 1], F32)
            gT = cpool.tile([E, BC], F32R)                 # gates, expert-major
            wgf = cpool.tile([128, DC, E], F32)            # fp32 gating weights
            ident = cpool.tile([128, 128], F32)

            make_identity(nc, ident[:, :])

            # loads ordered by first use: wg + x (gating) split across the SP
            # and ACT HWDGE queues, then W1 by pair interleaved with the small
            # tensors so pair 0 lands as early as possible
            nc.sync.dma_start(out=wgf[:, :, :],
                              in_=wg.bitcast(F32).rearrange("(c p) e -> p c e", p=128))
            for c in range(DC):
                eng = nc.sync if c < 3 else nc.scalar
                eng.dma_start(out=xsb[:, c, :], in_=xT[128 * c:128 * (c + 1), :])
            w1v = w1p.rearrange("(c q) h -> q c h", q=128)

            def load_w1(p):
                nc.sync.dma_start(out=w1sb[:, :, HP * p:HP * (p + 1)],
                                  in_=w1v[:, :, HP * p:HP * (p + 1)])

            w2tiles = {}
            ctx_w2 = tc.tile_pool(name="w2pool", bufs=3)
            w2pool = ctx_w2.__enter__()

            def load_w2(t, p):
                # W2 on the SP queue (not ACT: transfers there block the
                # latency-critical relu chain); one DMA per pair
                w2 = w2pool.tile([128, HCH, D], F32R, tag="w2", name="w2t")
                nc.sync.dma_start(
                    out=w2[:, :, :],
                    in_=w2p[HP * p:HP * (p + 1), :].rearrange(
                        "(m q) d -> q m d", q=128))
                w2tiles[(t, p)] = w2

            load_w1(0)
            nc.sync.dma_start(out=b1sb[:, :], in_=b1p[:, :])
            load_w1(1)
            load_w2(0, 0)
            load_w1(2)
            load_w2(0, 1)
            nc.sync.dma_start(out=b2sb[:, :], in_=b2[:, :])
            load_w1(3)
            load_w2(0, 2)
            for c in range(DC):
                nc.sync.dma_start(out=wosb[:, c, :], in_=wo[128 * c:128 * (c + 1), :])
            nc.sync.dma_start(out=bosb[:, :], in_=bo[:, :])
            for p in range(4, NPAIR):
                load_w1(p)
                load_w2(0, p - 1)
            load_w2(0, NPAIR - 1)

            # gates round-trip through DRAM; gate-broadcast tiles are read
            # back with a partition-step-0 DMA (POOL partition_broadcast is
            # HW-limited to out-base-partition 0)
            gdram = cpool.tile([E, BC], F32R, space="DRAM")
            gdram_ap = gdram
            gb_pre = {}
            gbpool = ctx_gb = tc.tile_pool(name="gbpool", bufs=2)
            gbpool = ctx_gb.__enter__()

            def load_gb(t, p):
                # gb[:,0,:] = gate(e0) broadcast over partitions; [:,1,:] = e1
                gb = gbpool.tile([128, 2, NT], F32R, tag="gb", name="gb")
                gb_src = bass.AP(tensor=gdram.tensor,
                                 offset=2 * p * BC + NT * t,
                                 ap=[[0, 128], [BC, 2], [1, NT]])
                nc.gpsimd.dma_start(out=gb[:, :, :], in_=gb_src)
                return gb

            # ---------------- gating pass (128-token subtiles) ----------------
            with tc.tile_pool(name="gps", bufs=2, space="PSUM") as gps, \
                 tc.tile_pool(name="gtp", bufs=2, space="PSUM") as gtp, \
                 tc.tile_pool(name="gsb", bufs=3) as gsb, \
                 tc.tile_pool(name="xgpool", bufs=2) as xgpool:
                xTv32 = xT.bitcast(F32).rearrange("(c q) b -> q c b", q=128)
                for i in range(BC // 128):
                    if i * 128 % NT == 0 and i > 0:
                        # flush finished token-half of the gates to DRAM early
                        lo = i * 128 - NT
                        nc.gpsimd.dma_start(out=gdram_ap[:, lo:lo + NT],
                                            in_=gT[:, lo:lo + NT])
                        if lo == 0:
                            gb_pre[(0, 0)] = load_gb(0, 0)
                            gb_pre[(0, 1)] = load_gb(0, 1)
                    ts = slice(128 * i, 128 * (i + 1))
                    # fp32-typed copies so the logits matmul runs in exact fp32
                    # (top-12 selection then matches the fp32 reference)
                    xg = xgpool.tile([128, DC, 128], F32, tag="xg", name="xg")
                    nc.gpsimd.dma_start(out=xg[:, :, :], in_=xTv32[:, :, ts])
                    lg_ps = gps.tile([128, E], F32, tag="lg")
                    for c in range(DC):
                        nc.tensor.matmul(lg_ps[:, :], xg[:, c, :], wgf[:, c, :],
                                         start=(c == 0), stop=(c == DC - 1))
                    lg = gsb.tile([128, E], F32, tag="lg_sb")
                    nc.vector.tensor_copy(lg[:, :], lg_ps[:, :])
                    # top-8 values, then values 9..16 after masking them out
                    t8a = gsb.tile([128, 8], F32, tag="t8a")
                    nc.vector.max(t8a[:, :], lg[:, :])
                    l2 = gsb.tile([128, E], F32, tag="l2")
                    nc.vector.match_replace(l2[:, :], t8a[:, :], lg[:, :], NEG_BIG)
                    t8b = gsb.tile([128, 8], F32, tag="t8b")
                    nc.vector.max(t8b[:, :], l2[:, :])
                    # softmax over entries >= 12th-largest (t8b[:,3])
                    e16 = gsb.tile([128, E], F32, tag="e16")
                    nc.scalar.activation(e16[:, :], lg[:, :],
                                         mybir.ActivationFunctionType.Exp)
                    em = gsb.tile([128, E], F32, tag="em")
                    ssum = gsb.tile([128, 1], F32, tag="ssum")
                    nc.vector.scalar_tensor_tensor(
                        out=em[:, :], in0=lg[:, :], scalar=t8b[:, 3:4],
                        in1=e16[:, :], op0=mybir.AluOpType.is_ge,
                        op1=mybir.AluOpType.mult, accum_out=ssum[:, :])
                    rinv = gsb.tile([128, 1], F32, tag="rinv")
                    nc.vector.reciprocal(rinv[:, :], ssum[:, :])
                    g = gsb.tile([128, E], F32, tag="g")
                    nc.vector.tensor_scalar_mul(g[:, :], em[:, :], rinv[:, :])
                    # transpose to expert-major gT[16, tokens]
                    gt_ps = gtp.tile([E, 128], F32, tag="gt")
                    nc.tensor.transpose(gt_ps[:, :], g[:, :], ident[:, :])
                    nc.vector.tensor_copy(gT[:, ts], gt_ps[:, :])
            if DEBUG:
                nc.sync.dma_start(out=dbg_gT[:, :], in_=gT[:, :].bitcast(F32))

            nc.gpsimd.dma_start(out=gdram_ap[:, BC - NT:], in_=gT[:, BC - NT:])

            # ---------------- main loop ----------------
            with tc.tile_pool(name="moeps", bufs=DC, space="PSUM") as moeps, \
                 tc.tile_pool(name="hps", bufs=2, space="PSUM") as hps, \
                 tc.tile_pool(name="gstpool", bufs=2) as gstpool, \
                 tc.tile_pool(name="hgpool", bufs=(20 if DEBUG else 22)) as hgpool, \
                 tc.tile_pool(name="opool", bufs=2) as opool:
                for t in range(NTILES):
                    ts = slice(NT * t, NT * (t + 1))
                    # one PSUM tile per d-chunk: a single big tile would put a
                    # false tile-level WAR between chunk c's drain (DVE read)
                    # and chunk c+1's accumulation (PE write)
                    moe = [moeps.tile([128, NT], F32, tag="moe", name="moe")
                           for _ in range(DC)]
                    w2t = [None] * NPAIR
                    hg = [[None] * HCH for _ in range(NPAIR)]

                    def stage1(p, ts=ts, moe=moe, w2t=w2t, hg=hg, t=t):
                        w2t[p] = w2tiles.pop((t, p), None)
                        if w2t[p] is None:
                            load_w2(t, p)
                            w2t[p] = w2tiles.pop((t, p))
                        gb = gb_pre.pop((t, p), None)
                        if gb is None:
                            gb = load_gb(t, p)
                        if DEBUG and t == 0 and p == 0:
                            nc.sync.dma_start(out=dbg_gb[:, :, :], in_=gb[:, :, :].bitcast(F32))
                        for m in range(HCH):
                            hcol = HP * p + 128 * m
                            hps_t = hps.tile([128, NT], F32, tag="h")
                            for c in range(DC):
                                nc.tensor.matmul(hps_t[:, :],
                                                 w1sb[:, c, hcol:hcol + 128],
                                                 xsb[:, c, ts],
                                                 start=(c == 0), stop=(c == DC - 1))
                            # relu(u + b1) in-place in PSUM, then gate-multiply to SBUF
                            nc.scalar.activation(hps_t[:, :], hps_t[:, :],
                                                 mybir.ActivationFunctionType.Relu,
                                                 bias=b1sb[:, hcol // 128:hcol // 128 + 1])
                            if DEBUG and t == 0 and p == 0 and m == 0:
                                dbg_h_sb = gstpool.tile([128, NT], F32, tag="dbg", name="dbg_h_sb")
                                nc.vector.tensor_copy(dbg_h_sb[:, :], hps_t[:, :])
                                nc.sync.dma_start(out=dbg_h[:, :], in_=dbg_h_sb[:, :])
                            hg[p][m] = hgpool.tile([128, NT], F32R, tag="hg", name="hg")
                            if m == 1:
                                # mixed chunk: parts 0:64 are e0's h[128:192],
                                # parts 64:128 are e1's h[0:64]
                                nc.vector.tensor_tensor(
                                    out=hg[p][m][0:64, :], in0=hps_t[0:64, :],
                                    in1=gb[0:64, 0, :].bitcast(F32),
                                    op=mybir.AluOpType.mult)
                                nc.vector.tensor_tensor(
                                    out=hg[p][m][64:128, :], in0=hps_t[64:128, :],
                                    in1=gb[64:128, 1, :].bitcast(F32),
                                    op=mybir.AluOpType.mult)
                            else:
                                nc.vector.tensor_tensor(
                                    out=hg[p][m][:, :], in0=hps_t[:, :],
                                    in1=gb[:, 0 if m == 0 else 1, :].bitcast(F32),
                                    op=mybir.AluOpType.mult)
                            if DEBUG and t == 0 and p == 0 and m == 0:
                                nc.sync.dma_start(out=dbg_hg[:, :], in_=hg[p][m][:, :].bitcast(F32))

                    def stage2(p, moe=moe, w2t=w2t, hg=hg, ts=ts, close=False):
                        if not close:
                            # m-outer: the first 12 matmuls need only hg m0/m1,
                            # giving hg m2's relu+mult chain extra cover
                            for m in range(HCH):
                                for c in range(DC):
                                    nc.tensor.matmul(moe[c][:, :],
                                                     w2t[p][:, m, 128 * c:128 * (c + 1)],
                                                     hg[p][m][:, :],
                                                     start=(p == 0 and m == 0), stop=False)
                            return
                        for c in range(DC):
                            for m in range(HCH):
                                nc.tensor.matmul(moe[c][:, :],
                                                 w2t[p][:, m, 128 * c:128 * (c + 1)],
                                                 hg[p][m][:, :],
                                                 start=(p == 0 and m == 0), stop=False)
                            if close:
                                # b2 bias term closes this chunk's accumulation
                                nc.tensor.matmul(moe[c][:, :],
                                                 b2sb[:, 128 * c:128 * (c + 1)],
                                                 gT[:, ts], start=False, stop=True)
                                finish_chunk(c)
                                # head matmul trails two chunks behind so its
                                # relu+residual drain is already complete
                                if c >= 2:
                                    head_chunk(c - 2)
                        if close:
                            head_chunk(DC - 2)
                            head_chunk(DC - 1)

                    def finish_chunk(c, moe=moe, ts=ts):
                        # z = relu(moe) + x in one DVE op, overwriting x in place
                        if DEBUG and t == 0:
                            dbg_moe_sb = gstpool.tile([128, NT], F32, tag="dbg", name="dbg_moe_sb")
                            nc.vector.tensor_copy(dbg_moe_sb[:, :], moe[c][:, :])
                            nc.sync.dma_start(out=dbg_moe[:, c, :], in_=dbg_moe_sb[:, :])
                        nc.vector.scalar_tensor_tensor(
                            out=xsb[:, c, ts], in0=moe[c][:, :], scalar=0.0,
                            in1=xsb[:, c, ts].bitcast(F32),
                            op0=mybir.AluOpType.max, op1=mybir.AluOpType.add)

                    out_ps_box = [None]

                    def head_chunk(c, ts=ts):
                        if out_ps_box[0] is None:
                            out_ps_box[0] = hps.tile([O, NT], F32, tag="h",
                                                     name="out_ps")
                        nc.tensor.matmul(out_ps_box[0][:, :], wosb[:, c, :],
                                         xsb[:, c, ts],
                                         start=(c == 0), stop=(c == DC - 1))

                    # software pipeline: stage1(p+1) covers stage2(p) latency;
                    # the last pair closes each moe chunk so relu/residual/head
                    # drain per chunk while later chunks still accumulate
                    stage1(0)
                    for p in range(NPAIR):
                        if p + 1 < NPAIR:
                            stage1(p + 1)
                        stage2(p, close=(p == NPAIR - 1))
                    out_ps = out_ps_box[0]
                    osb = opool.tile([O, NT], F32, tag="osb")
                    nc.scalar.activation(osb[:, :], out_ps[:, :],
                                         mybir.ActivationFunctionType.Identity,
                                         bias=bosb[:, :])
                    nc.sync.dma_start(out=outT[:, ts], in_=osb[:, :])
            ctx_gb.__exit__(None, None, None)
            ctx_w2.__exit__(None, None, None)

    nc.compile()
    return nc


def _pack_core_inputs(x, Wg, W1, b1, W2, b2, Wo, bo, c4):
    """Per-core input dict for one modality's weights + 1024-token slice."""
    f = np.float32
    tok = slice(BC * c4, BC * (c4 + 1))
    return {
        "xT": np.ascontiguousarray(np.asarray(x[tok], f).T),
        "w1p": np.ascontiguousarray(np.asarray(W1, f).transpose(1, 0, 2).reshape(D, E * H)),
        "w2p": np.ascontiguousarray(np.asarray(W2, f).reshape(E * H, D)),
        "b1p": np.ascontiguousarray(np.asarray(b1, f).reshape(-1).reshape(E * H // 128, 128).T),
        "b2": np.ascontiguousarray(np.asarray(b2, f)),
        "wg": np.ascontiguousarray(np.asarray(Wg, f)),
        "wo": np.ascontiguousarray(np.asarray(Wo, f)),
        "bo": np.ascontiguousarray(np.asarray(bo, f).reshape(O, 1)),
    }


def run_on_hw(inputs, trace=False, **kw):
    if "nc" not in _NC_CACHE:
        _NC_CACHE["nc"] = build_nc()
    nc = _NC_CACHE["nc"]
    in_maps = []
    for core in range(NCORES):
        i, c4 = divmod(core, 4)
        x = inputs["x0"] if i == 0 else inputs["x1"]
        in_maps.append(_pack_core_inputs(
            x, inputs["Wg"][i], inputs["W1"][i], inputs["b1"][i],
            inputs["W2"][i], inputs["b2"][i], inputs["Wo"][i], inputs["bo"][i], c4))
    res = run_bass_kernel_spmd(nc, in_maps, core_ids=list(range(NCORES)),
                               trace=trace, **kw)
    outs = []
    for i in range(2):
        outs.append(np.concatenate(
            [res.results[4 * i + c]["outT"].T for c in range(4)], axis=0))
    return (outs[0], outs[1]), res


def kernel(**inputs):
    (o0, o1), _ = run_on_hw(inputs)
    return (o0, o1)



# revision 2
# speedup vs baseline: 2.1094x; 2.1094x over previous
"""Trainium2 Bass kernel for nn_ClassifierGuided (2-modality top-12-of-16 MoE classifier).

Sharding: pure data-parallel over tokens. 2 modalities x 4096 tokens = 8192
tokens; each of the 8 cores owns 1024 tokens of one modality (cores 0-3 ->
modality 0, cores 4-7 -> modality 1) and that modality's full weights.
Dense-eval MoE (all 16 experts computed, sparse gates applied), so no
all-to-all is needed.

v2: expert matmuls run in fp8 (e4m3) with the DoubleRow perf mode (256-row
contraction per instruction at 0.5 cycles/row => 4x the fp32r rate). Experts
are processed in QUADS of 4 (768 h-dims = 3 DoubleRow chunk-pairs). The x
activations / gating / residual / head run in bf16 (exact f32 PSUM
accumulation); gates are quantized to fp8 and broadcast across partitions via
a DRAM roundtrip with partition-step-0 DMAs, laid out so every gate-multiply
is a full [128,512] op.

Engine split: PE = matmuls; Act = relu+bias (PSUM->SBUF f32) + gating exp;
Pool = gate-mults (fp8 out) + moe drain (relu+residual); DVE = top-k gating
chain + head bias.
"""
import sys

sys.path.insert(0, "/opt/trn_rl_repo")

import numpy as np
import ml_dtypes

import concourse.bass as bass
import concourse.mybir as mybir
import concourse.tile as tile
from concourse import bacc
from concourse.bass_utils import run_bass_kernel_spmd
from concourse.masks import make_identity

# ---- problem sizes (hardcoded per the harness contract) ----
B = 4096           # tokens per modality
D = 768            # model dim
E = 16             # experts
H = 192            # expert hidden
O = 101            # classifier out
KTOP = 12          # top-k experts
NCORES = 8
BC = B // 4        # 1024 tokens per core
DC = D // 128      # 6 d-chunks
NT = 512           # token tile (matmul moving dim)
NTILES = BC // NT  # 2
NQ = 4             # expert quads (4 experts each)
QH = 4 * H         # 768 h-dims per quad = 6 chunks of 128
QCH = QH // 128    # 6 h-chunks per quad
EH128 = E * H // 128  # 24 total h-chunks
F32 = mybir.dt.float32
F32R = mybir.dt.float32r
BF16 = mybir.dt.bfloat16
F8 = mybir.dt.float8e4
DR = mybir.MatmulPerfMode.DoubleRow
NEG_BIG = -1.0e30

_NC_CACHE = {}


def build_nc():
    nc = bacc.Bacc("TRN2", target_bir_lowering=False, debug=False,
                   num_devices=NCORES)

    # ---- DRAM I/O (per-core views; host pre-packs) ----
    xbf_d = nc.dram_tensor("xbf", [128, DC, BC], BF16, kind="ExternalInput").ap()
    x8_d = nc.dram_tensor("x8", [128, DC, BC], F8, kind="ExternalInput").ap()
    w1_d = nc.dram_tensor("w1", [128, DC, E * H], F8, kind="ExternalInput").ap()
    w2_d = nc.dram_tensor("w2", [128, EH128, D], F8, kind="ExternalInput").ap()
    b1_d = nc.dram_tensor("b1", [128, EH128], F32, kind="ExternalInput").ap()
    b2_d = nc.dram_tensor("b2", [E, D], F8, kind="ExternalInput").ap()
    wg_d = nc.dram_tensor("wg", [128, DC, E], BF16, kind="ExternalInput").ap()
    wo_d = nc.dram_tensor("wo", [128, DC, O], BF16, kind="ExternalInput").ap()
    bo_d = nc.dram_tensor("bo", [O, 1], F32, kind="ExternalInput").ap()
    outT = nc.dram_tensor("outT", [O, BC], F32, kind="ExternalOutput").ap()

    with tile.TileContext(nc) as tc:
        with tc.tile_pool(name="const", bufs=1) as cpool:
            # resident SBUF tensors
            xbf = cpool.tile([128, DC, BC], BF16)      # x (bf16); later holds z
            x8 = cpool.tile([128, DC, BC], F8)         # x (fp8) for W1 matmuls
            w1sb = cpool.tile([128, DC, E * H], F8)
            w2sb = cpool.tile([128, EH128, D], F8)
            b1sb = cpool.tile([128, EH128], F32)
            b2sb = cpool.tile([E, D], F8)
            wgb = cpool.tile([128, DC, E], BF16)
            wob = cpool.tile([128, DC, O], BF16)
            bosb = cpool.tile([O, 1], F32)
            g8 = cpool.tile([E, BC], F8)               # fp8 gates, expert-major
            ident = cpool.tile([128, 128], F32)
            gdram = cpool.tile([E, BC], F8, space="DRAM")

            make_identity(nc, ident[:, :])

            # loads ordered by first use. SP queue: gating weights + fp8
            # tensors; Act queue: bf16 x + W2 + Wo (all waitless so the Act
            # engine's relu chain is never blocked).
            nc.sync.dma_start(out=wgb[:, :, :], in_=wg_d)
            for t in range(NTILES):
                ts = slice(NT * t, NT * (t + 1))
                nc.scalar.dma_start(out=xbf[:, :, ts], in_=xbf_d[:, :, ts])
            nc.sync.dma_start(out=x8[:, :, :], in_=x8_d)
            for q in range(NQ):
                hs = slice(QH * q, QH * (q + 1))
                nc.sync.dma_start(out=w1sb[:, :, hs], in_=w1_d[:, :, hs])
                nc.scalar.dma_start(out=w2sb[:, 6 * q:6 * (q + 1), :],
                                    in_=w2_d[:, 6 * q:6 * (q + 1), :])
            nc.sync.dma_start(out=b1sb[:, :], in_=b1_d)
            nc.sync.dma_start(out=b2sb[:, :], in_=b2_d)
            nc.scalar.dma_start(out=wob[:, :, :], in_=wo_d)
            nc.sync.dma_start(out=bosb[:, :], in_=bo_d)

            # gb6 tiles: per (tile, quad) a [128, QCH, NT] fp8 tile where
            # gb[:, j, :] carries the gate of the expert owning h-chunk j,
            # with the two mixed chunks split at partition 64. All 8 resident.
            gb_tiles = {}

            def load_gb6(t, q):
                gb = cpool.tile([128, QCH, NT], F8, tag="gb6", name="gb6")
                # expert k of the quad covers partition-halves 3k..3k+2:
                #   k=0: all of j=0 + lower half of j=1
                #   k=1: upper half of j=1 + all of j=2
                #   k=2: all of j=3 + lower half of j=4
                #   k=3: upper half of j=4 + all of j=5
                for k in range(4):
                    e = 4 * q + k
                    off = e * BC + NT * t
                    full_j = [0, 2, 3, 5][k]
                    nc.sync.dma_start(
                        out=gb[:, full_j, :],
                        in_=bass.AP(tensor=gdram.tensor, offset=off,
                                    ap=[[0, 128], [1, NT]]))
                    half_j = 1 if k < 2 else 4
                    plo, phi = (0, 64) if k % 2 == 0 else (64, 128)
                    nc.sync.dma_start(
                        out=gb[plo:phi, half_j, :],
                        in_=bass.AP(tensor=gdram.tensor, offset=off,
                                    ap=[[0, 64], [1, NT]]))
                gb_tiles[(t, q)] = gb

            # ---------------- gating pass (128-token subtiles) ----------------
            # logits accumulate exactly in f32 PSUM from bf16 inputs; top-12
            # selection + softmax runs in f32 on DVE, gates stored as fp8.
            with tc.tile_pool(name="gps", bufs=2, space="PSUM") as gps, \
                 tc.tile_pool(name="gtp", bufs=2, space="PSUM") as gtp, \
                 tc.tile_pool(name="gsb", bufs=3) as gsb:
                for i in range(BC // 128):
                    ts = slice(128 * i, 128 * (i + 1))
                    lg_ps = gps.tile([128, E], F32, tag="lg")
                    for c in range(DC):
                        nc.tensor.matmul(lg_ps[:, :], xbf[:, c, ts], wgb[:, c, :],
                                         start=(c == 0), stop=(c == DC - 1))
                    lg = gsb.tile([128, E], F32, tag="lg_sb")
                    nc.vector.tensor_copy(lg[:, :], lg_ps[:, :])
                    # top-8 values, then values 9..16 after masking them out
                    t8a = gsb.tile([128, 8], F32, tag="t8a")
                    nc.vector.max(t8a[:, :], lg[:, :])
                    l2 = gsb.tile([128, E], F32, tag="l2")
                    nc.vector.match_replace(l2[:, :], t8a[:, :], lg[:, :], NEG_BIG)
                    t8b = gsb.tile([128, 8], F32, tag="t8b")
                    nc.vector.max(t8b[:, :], l2[:, :])
                    # softmax over entries >= 12th-largest (t8b[:,3])
                    e16 = gsb.tile([128, E], F32, tag="e16")
                    nc.scalar.activation(e16[:, :], lg[:, :],
                                         mybir.ActivationFunctionType.Exp)
                    em = gsb.tile([128, E], F32, tag="em")
                    ssum = gsb.tile([128, 1], F32, tag="ssum")
                    nc.vector.scalar_tensor_tensor(
                        out=em[:, :], in0=lg[:, :], scalar=t8b[:, 3:4],
                        in1=e16[:, :], op0=mybir.AluOpType.is_ge,
                        op1=mybir.AluOpType.mult, accum_out=ssum[:, :])
                    rinv = gsb.tile([128, 1], F32, tag="rinv")
                    nc.vector.reciprocal(rinv[:, :], ssum[:, :])
                    g = gsb.tile([128, E], F32, tag="g")
                    nc.vector.tensor_scalar_mul(g[:, :], em[:, :], rinv[:, :])
                    # transpose to expert-major, quantize to fp8
                    gt_ps = gtp.tile([E, 128], F32, tag="gt")
                    nc.tensor.transpose(gt_ps[:, :], g[:, :], ident[:, :])
                    nc.vector.tensor_copy(g8[:, ts], gt_ps[:, :])
                    if 128 * (i + 1) % NT == 0:
                        # flush this token-half of the gates and start the
                        # partition-broadcast reads for its quads
                        t = (128 * (i + 1)) // NT - 1
                        fs = slice(NT * t, NT * (t + 1))
                        nc.sync.dma_start(out=gdram[:, fs], in_=g8[:, fs])
                        for q in range(NQ):
                            load_gb6(t, q)

            # ---------------- main loop ----------------
            with tc.tile_pool(name="moeps", bufs=DC, space="PSUM") as moeps, \
                 tc.tile_pool(name="hps", bufs=2, space="PSUM") as hps, \
                 tc.tile_pool(name="hsb", bufs=4) as hsb, \
                 tc.tile_pool(name="hgpool", bufs=3) as hgpool, \
                 tc.tile_pool(name="opool", bufs=2) as opool:
                for t in range(NTILES):
                    ts = slice(NT * t, NT * (t + 1))
                    # one PSUM tile per d-chunk; single big tile would add a
                    # false WAR between drain and later accumulation
                    moe = [moeps.tile([128, NT], F32, tag="moe", name="moe")
                           for _ in range(DC)]
                    hg = [None] * NQ

                    def stage1(q, ts=ts, hg=hg, t=t):
                        gb = gb_tiles[(t, q)]
                        hgq = hgpool.tile([128, QCH, NT], F8, tag="hg", name="hg")
                        hg[q] = hgq
                        for j in range(QCH):
                            m = QCH * q + j          # global h-chunk
                            hcol = 128 * m
                            hps_t = hps.tile([128, NT], F32, tag="h", name="h")
                            for c2 in range(DC // 2):
                                nc.tensor.matmul(
                                    hps_t[:, :],
                                    w1sb[:, 2 * c2:2 * c2 + 2, hcol:hcol + 128],
                                    x8[:, 2 * c2:2 * c2 + 2, ts],
                                    start=(c2 == 0), stop=(c2 == DC // 2 - 1),
                                    perf_mode=DR)
                            # relu(u + b1) PSUM -> SBUF f32 on Act
                            hf = hsb.tile([128, NT], F32, tag="hf", name="hf")
                            nc.scalar.activation(hf[:, :], hps_t[:, :],
                                                 mybir.ActivationFunctionType.Relu,
                                                 bias=b1sb[:, m:m + 1])
                            # gate-multiply -> fp8 on Pool
                            nc.gpsimd.tensor_tensor(
                                out=hgq[:, j, :], in0=hf[:, :],
                                in1=gb[:, j, :], op=mybir.AluOpType.mult)

                    def stage2(q, moe=moe, hg=hg, ts=ts, close=False):
                        if not close:
                            for j2 in range(QCH // 2):
                                m2 = (QCH // 2) * q + j2   # global chunk-pair
                                for c in range(DC):
                                    nc.tensor.matmul(
                                        moe[c][:, :],
                                        w2sb[:, 2 * m2:2 * m2 + 2,
                                             128 * c:128 * (c + 1)],
                                        hg[q][:, 2 * j2:2 * j2 + 2, :],
                                        start=(q == 0 and j2 == 0), stop=False,
                                        perf_mode=DR)
                            return
                        for c in range(DC):
                            for j2 in range(QCH // 2):
                                m2 = (QCH // 2) * q + j2
                                nc.tensor.matmul(
                                    moe[c][:, :],
                                    w2sb[:, 2 * m2:2 * m2 + 2,
                                         128 * c:128 * (c + 1)],
                                    hg[q][:, 2 * j2:2 * j2 + 2, :],
                                    start=False, stop=False, perf_mode=DR)
                            # b2 bias term closes this chunk's accumulation
                            nc.tensor.matmul(moe[c][:, :],
                                             b2sb[:, 128 * c:128 * (c + 1)],
                                             g8[:, ts], start=False, stop=True)
                            finish_chunk(c)
                            # head matmul trails two chunks behind so its
                            # relu+residual drain is already complete
                            if c >= 2:
                                head_chunk(c - 2)
                        head_chunk(DC - 2)
                        head_chunk(DC - 1)

                    def finish_chunk(c, moe=moe, ts=ts):
                        # z = relu(moe) + x in one Pool op, bf16 in place of x
                        nc.gpsimd.scalar_tensor_tensor(
                            out=xbf[:, c, ts], in0=moe[c][:, :], scalar=0.0,
                            in1=xbf[:, c, ts], op0=mybir.AluOpType.max,
                            op1=mybir.AluOpType.add)

                    out_ps_box = [None]

                    def head_chunk(c, ts=ts):
                        if out_ps_box[0] is None:
                            out_ps_box[0] = hps.tile([O, NT], F32, tag="h",
                                                     name="out_ps")
                        nc.tensor.matmul(out_ps_box[0][:, :], wob[:, c, :],
                                         xbf[:, c, ts],
                                         start=(c == 0), stop=(c == DC - 1))

                    # software pipeline: interleave stage1(q+1) h-chunks with
                    # stage2(q) so the relu/gate chain is always covered
                    stage1(0)
                    for q in range(NQ):
                        if q + 1 < NQ:
                            stage1(q + 1)
                        stage2(q, close=(q == NQ - 1))
                    out_ps = out_ps_box[0]
                    osb = opool.tile([O, NT], F32, tag="osb")
                    nc.vector.tensor_scalar_add(osb[:, :], out_ps[:, :],
                                                bosb[:, 0:1])
                    nc.sync.dma_start(out=outT[:, ts], in_=osb[:, :])

    nc.compile()
    return nc


def _pack_core_inputs(x, Wg, W1, b1, W2, b2, Wo, bo, c4):
    """Per-core input dict for one modality's weights + 1024-token slice."""
    f = np.float32
    bf = ml_dtypes.bfloat16
    f8 = ml_dtypes.float8_e4m3
    tok = slice(BC * c4, BC * (c4 + 1))
    xT = np.asarray(x[tok], f).T                       # [D, BC]
    xr = np.ascontiguousarray(
        xT.reshape(DC, 128, BC).transpose(1, 0, 2))    # [128, DC, BC]
    w1f = np.asarray(W1, f).transpose(1, 0, 2).reshape(D, E * H)
    w2f = np.asarray(W2, f).reshape(E * H, D)
    return {
        "xbf": xr.astype(bf),
        "x8": xr.astype(f8),
        "w1": np.ascontiguousarray(
            w1f.reshape(DC, 128, E * H).transpose(1, 0, 2)).astype(f8),
        "w2": np.ascontiguousarray(
            w2f.reshape(EH128, 128, D).transpose(1, 0, 2)).astype(f8),
        "b1": np.ascontiguousarray(
            np.asarray(b1, f).reshape(EH128, 128).T),
        "b2": np.asarray(b2, f).astype(f8),
        "wg": np.ascontiguousarray(
            np.asarray(Wg, f).reshape(DC, 128, E).transpose(1, 0, 2)).astype(bf),
        "wo": np.ascontiguousarray(
            np.asarray(Wo, f).reshape(DC, 128, O).transpose(1, 0, 2)).astype(bf),
        "bo": np.ascontiguousarray(np.asarray(bo, f).reshape(O, 1)),
    }


def run_on_hw(inputs, trace=False, **kw):
    if "nc" not in _NC_CACHE:
        _NC_CACHE["nc"] = build_nc()
    nc = _NC_CACHE["nc"]
    in_maps = []
    for core in range(NCORES):
        i, c4 = divmod(core, 4)
        x = inputs["x0"] if i == 0 else inputs["x1"]
        in_maps.append(_pack_core_inputs(
            x, inputs["Wg"][i], inputs["W1"][i], inputs["b1"][i],
            inputs["W2"][i], inputs["b2"][i], inputs["Wo"][i], inputs["bo"][i], c4))
    res = run_bass_kernel_spmd(nc, in_maps, core_ids=list(range(NCORES)),
                               trace=trace, **kw)
    outs = []
    for i in range(2):
        outs.append(np.concatenate(
            [res.results[4 * i + c]["outT"].T for c in range(4)], axis=0))
    return (outs[0], outs[1]), res


def kernel(**inputs):
    (o0, o1), _ = run_on_hw(inputs)
    return (o0, o1)


# revision 3
# speedup vs baseline: 2.3710x; 1.1240x over previous
"""Trainium2 Bass kernel for nn_ClassifierGuided (2-modality top-12-of-16 MoE classifier).

Sharding: pure data-parallel over tokens. 2 modalities x 4096 tokens = 8192
tokens; each of the 8 cores owns 1024 tokens of one modality (cores 0-3 ->
modality 0, cores 4-7 -> modality 1) and that modality's full weights.
Dense-eval MoE (all 16 experts computed, sparse gates applied), so no
all-to-all is needed.

v3: expert matmuls run in fp8 (e4m3) with the DoubleRow perf mode (256-row
contraction per instruction at 0.5 cycles/row => 4x the fp32r rate). Experts
are processed in QUADS of 4 (768 h-dims = 3 DoubleRow chunk-pairs). The x
activations / gating / residual / head run in bf16 (exact f32 PSUM
accumulation); gates are quantized to fp8 and broadcast across partitions via
a DRAM roundtrip with partition-step-0 DMAs, laid out per h-chunk so every
gate-multiply is a full [128,512] op. All gb tiles are resident (no reuse
waits). The main loop is a flat software pipeline over (tile, quad): stage1
h-chunks of quad k+1 interleave with stage2 matmul groups of quad k so the PE
never waits on the Act-paced relu chain.

Engine split: PE = matmuls; Act = relu+bias (PSUM->SBUF f32) + gating exp;
Pool = gate-mults (fp8 out, SBUF only); DVE = top-k gating chain + moe drain
(relu+residual, PSUM) + head bias.
"""
import sys

sys.path.insert(0, "/opt/trn_rl_repo")

import numpy as np
import ml_dtypes

import concourse.bass as bass
import concourse.mybir as mybir
import concourse.tile as tile
from concourse import bacc
from concourse.bass_utils import run_bass_kernel_spmd
from concourse.masks import make_identity

# ---- problem sizes (hardcoded per the harness contract) ----
B = 4096           # tokens per modality
D = 768            # model dim
E = 16             # experts
H = 192            # expert hidden
O = 101            # classifier out
KTOP = 12          # top-k experts
NCORES = 8
BC = B // 4        # 1024 tokens per core
DC = D // 128      # 6 d-chunks
NT = 512           # token tile (matmul moving dim)
NTILES = BC // NT  # 2
NQ = 4             # expert quads (4 experts each)
QH = 4 * H         # 768 h-dims per quad = 6 chunks of 128
QCH = QH // 128    # 6 h-chunks per quad
EH128 = E * H // 128  # 24 total h-chunks
F32 = mybir.dt.float32
F32R = mybir.dt.float32r
BF16 = mybir.dt.bfloat16
F8 = mybir.dt.float8e4
DR = mybir.MatmulPerfMode.DoubleRow
NEG_BIG = -1.0e30

_NC_CACHE = {}


def build_nc():
    nc = bacc.Bacc("TRN2", target_bir_lowering=False, debug=False,
                   num_devices=NCORES)

    # ---- DRAM I/O (per-core views; host pre-packs) ----
    xbf_d = nc.dram_tensor("xbf", [128, DC, BC], BF16, kind="ExternalInput").ap()
    x8_d = nc.dram_tensor("x8", [128, DC, BC], F8, kind="ExternalInput").ap()
    w1_d = nc.dram_tensor("w1", [128, DC, E * H], F8, kind="ExternalInput").ap()
    w2_d = nc.dram_tensor("w2", [128, EH128, D], F8, kind="ExternalInput").ap()
    b1_d = nc.dram_tensor("b1", [128, EH128], F32, kind="ExternalInput").ap()
    b2_d = nc.dram_tensor("b2", [E, D], F8, kind="ExternalInput").ap()
    wg_d = nc.dram_tensor("wg", [128, DC, E], BF16, kind="ExternalInput").ap()
    wo_d = nc.dram_tensor("wo", [128, DC, O], BF16, kind="ExternalInput").ap()
    bo_d = nc.dram_tensor("bo", [O, 1], F32, kind="ExternalInput").ap()
    outT = nc.dram_tensor("outT", [O, BC], F32, kind="ExternalOutput").ap()

    with tile.TileContext(nc) as tc:
        with tc.tile_pool(name="const", bufs=1) as cpool, \
             tc.tile_pool(name="gbpool", bufs=8) as gbpool:
            # resident SBUF tensors
            xbf = cpool.tile([128, DC, BC], BF16)      # x (bf16); later holds z
            x8 = cpool.tile([128, DC, BC], F8)         # x (fp8) for W1 matmuls
            w1sb = cpool.tile([128, DC, E * H], F8)
            w2sb = cpool.tile([128, EH128, D], F8)
            b1sb = cpool.tile([128, EH128], F32)
            b2sb = cpool.tile([E, D], F8)
            wgb = cpool.tile([128, DC, E], BF16)
            wob = cpool.tile([128, DC, O], BF16)
            bosb = cpool.tile([O, 1], F32)
            g8 = cpool.tile([E, BC], F8)               # fp8 gates, expert-major
            ident = cpool.tile([128, 128], F32)
            gdram = cpool.tile([E, BC], F8, space="DRAM")

            make_identity(nc, ident[:, :])

            # loads ordered by first use. SP queue: gating weights + fp8
            # tensors + gate broadcasts; Act queue: bf16 x + W2 + Wo (all
            # waitless so the Act engine's relu chain is never blocked).
            nc.sync.dma_start(out=wgb[:, :, :], in_=wg_d)
            for p in range(4):
                ts = slice(256 * p, 256 * (p + 1))
                nc.scalar.dma_start(out=xbf[:, :, ts], in_=xbf_d[:, :, ts])
            nc.sync.dma_start(out=x8[:, :, :], in_=x8_d)
            for q in range(NQ):
                hs = slice(QH * q, QH * (q + 1))
                nc.sync.dma_start(out=w1sb[:, :, hs], in_=w1_d[:, :, hs])
                nc.scalar.dma_start(out=w2sb[:, 6 * q:6 * (q + 1), :],
                                    in_=w2_d[:, 6 * q:6 * (q + 1), :])
            nc.sync.dma_start(out=b1sb[:, :], in_=b1_d)
            nc.sync.dma_start(out=b2sb[:, :], in_=b2_d)
            nc.scalar.dma_start(out=wob[:, :, :], in_=wo_d)
            nc.sync.dma_start(out=bosb[:, :], in_=bo_d)

            # gb tiles: per (tile, quad) a [128, QCH, NT] fp8 tile where
            # gb[:, j, :] carries the gate of the expert owning h-chunk j,
            # with the two mixed chunks split at partition 64. All 8 resident.
            gb_tiles = {}

            def load_gb6(t, q):
                gb = gbpool.tile([128, QCH, NT], F8, tag="gb6", name="gb6")
                off = 4 * q * BC + NT * t
                # experts (0,1) of the quad own full chunks j=0,2; (2,3) own
                # j=3,5; expert k covers partition-halves 3k..3k+2
                nc.sync.dma_start(
                    out=gb[:, 0:3:2, :],
                    in_=bass.AP(tensor=gdram.tensor, offset=off,
                                ap=[[0, 128], [BC, 2], [1, NT]]))
                nc.sync.dma_start(
                    out=gb[:, 3:6:2, :],
                    in_=bass.AP(tensor=gdram.tensor, offset=off + 2 * BC,
                                ap=[[0, 128], [BC, 2], [1, NT]]))
                for k in range(4):
                    half_j = 1 if k < 2 else 4
                    plo, phi = (0, 64) if k % 2 == 0 else (64, 128)
                    nc.sync.dma_start(
                        out=gb[plo:phi, half_j, :],
                        in_=bass.AP(tensor=gdram.tensor, offset=off + k * BC,
                                    ap=[[0, 64], [1, NT]]))
                gb_tiles[(t, q)] = gb

            # ---------------- gating pass (128-token subtiles) ----------------
            # logits accumulate exactly in f32 PSUM from bf16 inputs; top-12
            # selection + softmax runs in f32 on DVE, gates stored as fp8.
            with tc.tile_pool(name="gps", bufs=2, space="PSUM") as gps, \
                 tc.tile_pool(name="gtp", bufs=2, space="PSUM") as gtp, \
                 tc.tile_pool(name="gsb", bufs=3) as gsb:
                for i in range(BC // 128):
                    ts = slice(128 * i, 128 * (i + 1))
                    lg_ps = gps.tile([128, E], F32, tag="lg")
                    for c in range(DC):
                        nc.tensor.matmul(lg_ps[:, :], xbf[:, c, ts], wgb[:, c, :],
                                         start=(c == 0), stop=(c == DC - 1))
                    lg = gsb.tile([128, E], F32, tag="lg_sb")
                    nc.vector.tensor_copy(lg[:, :], lg_ps[:, :])
                    # top-8 values, then values 9..16 after masking them out
                    t8a = gsb.tile([128, 8], F32, tag="t8a")
                    nc.vector.max(t8a[:, :], lg[:, :])
                    l2 = gsb.tile([128, E], F32, tag="l2")
                    nc.vector.match_replace(l2[:, :], t8a[:, :], lg[:, :], NEG_BIG)
                    t8b = gsb.tile([128, 8], F32, tag="t8b")
                    nc.vector.max(t8b[:, :], l2[:, :])
                    # softmax over entries >= 12th-largest (t8b[:,3])
                    e16 = gsb.tile([128, E], F32, tag="e16")
                    nc.scalar.activation(e16[:, :], lg[:, :],
                                         mybir.ActivationFunctionType.Exp)
                    em = gsb.tile([128, E], F32, tag="em")
                    ssum = gsb.tile([128, 1], F32, tag="ssum")
                    nc.vector.scalar_tensor_tensor(
                        out=em[:, :], in0=lg[:, :], scalar=t8b[:, 3:4],
                        in1=e16[:, :], op0=mybir.AluOpType.is_ge,
                        op1=mybir.AluOpType.mult, accum_out=ssum[:, :])
                    rinv = gsb.tile([128, 1], F32, tag="rinv")
                    nc.vector.reciprocal(rinv[:, :], ssum[:, :])
                    g = gsb.tile([128, E], F32, tag="g")
                    nc.vector.tensor_scalar_mul(g[:, :], em[:, :], rinv[:, :])
                    # transpose to expert-major, quantize to fp8
                    gt_ps = gtp.tile([E, 128], F32, tag="gt")
                    nc.tensor.transpose(gt_ps[:, :], g[:, :], ident[:, :])
                    nc.vector.tensor_copy(g8[:, ts], gt_ps[:, :])
                    if 128 * (i + 1) % NT == 0:
                        # flush this token-half of the gates and start the
                        # partition-broadcast reads for its quads
                        t = (128 * (i + 1)) // NT - 1
                        fs = slice(NT * t, NT * (t + 1))
                        nc.sync.dma_start(out=gdram[:, fs], in_=g8[:, fs])
                        for q in range(NQ):
                            load_gb6(t, q)

            # ---------------- main loop: flat (tile, quad) pipeline ----------
            with tc.tile_pool(name="moeps", bufs=DC, space="PSUM") as moeps, \
                 tc.tile_pool(name="hps", bufs=2, space="PSUM") as hps, \
                 tc.tile_pool(name="hsb", bufs=4) as hsb, \
                 tc.tile_pool(name="hgpool", bufs=3) as hgpool, \
                 tc.tile_pool(name="opool", bufs=2) as opool:

                moes = {}      # t -> list of 6 moe PSUM tiles
                hgs = {}       # (t, q) -> hg tile
                out_ps = {}    # t -> head PSUM tile

                def s1_chunk(t, q, j):
                    """One h-chunk: 3 DR matmuls + relu(Act) + gate-mult(Pool)."""
                    ts = slice(NT * t, NT * (t + 1))
                    m = QCH * q + j
                    hcol = 128 * m
                    hps_t = hps.tile([128, NT], F32, tag="h", name="h")
                    for c2 in range(DC // 2):
                        nc.tensor.matmul(
                            hps_t[:, :],
                            w1sb[:, 2 * c2:2 * c2 + 2, hcol:hcol + 128],
                            x8[:, 2 * c2:2 * c2 + 2, ts],
                            start=(c2 == 0), stop=(c2 == DC // 2 - 1),
                            perf_mode=DR)
                    hf = hsb.tile([128, NT], F32, tag="hf", name="hf")
                    nc.scalar.activation(hf[:, :], hps_t[:, :],
                                         mybir.ActivationFunctionType.Relu,
                                         bias=b1sb[:, m:m + 1])
                    nc.gpsimd.tensor_tensor(
                        out=hgs[(t, q)][:, j, :], in0=hf[:, :],
                        in1=gb_tiles[(t, q)][:, j, :], op=mybir.AluOpType.mult)

                def s1_alloc(t, q):
                    hgs[(t, q)] = hgpool.tile([128, QCH, NT], F8, tag="hg",
                                              name="hg")

                def s2_mm(t, q, j2, c):
                    m2 = (QCH // 2) * q + j2
                    nc.tensor.matmul(
                        moes[t][c][:, :],
                        w2sb[:, 2 * m2:2 * m2 + 2, 128 * c:128 * (c + 1)],
                        hgs[(t, q)][:, 2 * j2:2 * j2 + 2, :],
                        start=(q == 0 and j2 == 0), stop=False, perf_mode=DR)

                def finish_chunk(t, c):
                    # z = relu(moe) + x in one DVE op, bf16 in place of x
                    ts = slice(NT * t, NT * (t + 1))
                    nc.vector.scalar_tensor_tensor(
                        out=xbf[:, c, ts], in0=moes[t][c][:, :], scalar=0.0,
                        in1=xbf[:, c, ts], op0=mybir.AluOpType.max,
                        op1=mybir.AluOpType.add)

                def head_chunk(t, c):
                    ts = slice(NT * t, NT * (t + 1))
                    if t not in out_ps:
                        out_ps[t] = hps.tile([O, NT], F32, tag="h",
                                             name="out_ps")
                    nc.tensor.matmul(out_ps[t][:, :], wob[:, c, :],
                                     xbf[:, c, ts],
                                     start=(c == 0), stop=(c == DC - 1))

                def s2_groups(t, q, close):
                    """Yield stage2 work as 6 groups (to interleave with the
                    next quad's 6 s1 chunks)."""
                    ts = slice(NT * t, NT * (t + 1))
                    if not close:
                        # j2-major so the last hg chunk-pair is needed late
                        for j2 in range(QCH // 2):
                            for ch in range(2):
                                def grp(j2=j2, ch=ch):
                                    for c in range(3 * ch, 3 * ch + 3):
                                        s2_mm(t, q, j2, c)
                                yield grp
                        return
                    # close: j2=0,1 accumulation first (j2-major), then a
                    # c-major pass with the final pair + b2 + drain + head
                    def grp_a():
                        for c in range(DC):
                            s2_mm(t, q, 0, c)
                    yield grp_a

                    def grp_b():
                        for c in range(DC):
                            s2_mm(t, q, 1, c)
                    yield grp_b

                    for ch in range(4):
                        def grp_c(ch=ch):
                            cs = [(0, 1), (2,), (3, 4), (5,)][ch]
                            for c in cs:
                                s2_mm(t, q, 2, c)
                                nc.tensor.matmul(
                                    moes[t][c][:, :],
                                    b2sb[:, 128 * c:128 * (c + 1)],
                                    g8[:, ts], start=False, stop=True)
                                finish_chunk(t, c)
                                if c >= 2:
                                    head_chunk(t, c - 2)
                            if ch == 3:
                                head_chunk(t, DC - 2)
                                head_chunk(t, DC - 1)
                        yield grp_c

                def emit_out(t):
                    ts = slice(NT * t, NT * (t + 1))
                    osb = opool.tile([O, NT], F32, tag="osb")
                    nc.vector.tensor_scalar_add(osb[:, :], out_ps[t][:, :],
                                                bosb[:, 0:1])
                    nc.sync.dma_start(out=outT[:, ts], in_=osb[:, :])

                seq = [(t, q) for t in range(NTILES) for q in range(NQ)]
                for t in range(NTILES):
                    moes[t] = [moeps.tile([128, NT], F32, tag="moe", name="moe")
                               for _ in range(DC)]
                # prologue: first quad's stage1, un-interleaved
                s1_alloc(0, 0)
                for j in range(QCH):
                    s1_chunk(0, 0, j)
                for idx, (t, q) in enumerate(seq):
                    nxt = seq[idx + 1] if idx + 1 < len(seq) else None
                    groups = list(s2_groups(t, q, close=(q == NQ - 1)))
                    if nxt is not None:
                        s1_alloc(*nxt)
                        for j in range(QCH):
                            s1_chunk(nxt[0], nxt[1], j)
                            groups[j]()
                    else:
                        for grp in groups:
                            grp()
                    if q == NQ - 1:
                        emit_out(t)

    nc.compile()
    return nc


def _pack_core_inputs(x, Wg, W1, b1, W2, b2, Wo, bo, c4):
    """Per-core input dict for one modality's weights + 1024-token slice."""
    f = np.float32
    bf = ml_dtypes.bfloat16
    f8 = ml_dtypes.float8_e4m3
    tok = slice(BC * c4, BC * (c4 + 1))
    xT = np.asarray(x[tok], f).T                       # [D, BC]
    xr = np.ascontiguousarray(
        xT.reshape(DC, 128, BC).transpose(1, 0, 2))    # [128, DC, BC]
    w1f = np.asarray(W1, f).transpose(1, 0, 2).reshape(D, E * H)
    w2f = np.asarray(W2, f).reshape(E * H, D)
    return {
        "xbf": xr.astype(bf),
        "x8": xr.astype(f8),
        "w1": np.ascontiguousarray(
            w1f.reshape(DC, 128, E * H).transpose(1, 0, 2)).astype(f8),
        "w2": np.ascontiguousarray(
            w2f.reshape(EH128, 128, D).transpose(1, 0, 2)).astype(f8),
        "b1": np.ascontiguousarray(
            np.asarray(b1, f).reshape(EH128, 128).T),
        "b2": np.asarray(b2, f).astype(f8),
        "wg": np.ascontiguousarray(
            np.asarray(Wg, f).reshape(DC, 128, E).transpose(1, 0, 2)).astype(bf),
        "wo": np.ascontiguousarray(
            np.asarray(Wo, f).reshape(DC, 128, O).transpose(1, 0, 2)).astype(bf),
        "bo": np.ascontiguousarray(np.asarray(bo, f).reshape(O, 1)),
    }


def run_on_hw(inputs, trace=False, **kw):
    if "nc" not in _NC_CACHE:
        _NC_CACHE["nc"] = build_nc()
    nc = _NC_CACHE["nc"]
    in_maps = []
    for core in range(NCORES):
        i, c4 = divmod(core, 4)
        x = inputs["x0"] if i == 0 else inputs["x1"]
        in_maps.append(_pack_core_inputs(
            x, inputs["Wg"][i], inputs["W1"][i], inputs["b1"][i],
            inputs["W2"][i], inputs["b2"][i], inputs["Wo"][i], inputs["bo"][i], c4))
    res = run_bass_kernel_spmd(nc, in_maps, core_ids=list(range(NCORES)),
                               trace=trace, **kw)
    outs = []
    for i in range(2):
        outs.append(np.concatenate(
            [res.results[4 * i + c]["outT"].T for c in range(4)], axis=0))
    return (outs[0], outs[1]), res


def kernel(**inputs):
    (o0, o1), _ = run_on_hw(inputs)
    return (o0, o1)


# revision 6
# speedup vs baseline: 2.3825x; 1.0049x over previous
"""Trainium2 Bass kernel for nn_ClassifierGuided (2-modality top-12-of-16 MoE classifier).

Sharding: pure data-parallel over tokens. 2 modalities x 4096 tokens = 8192
tokens; each of the 8 cores owns 1024 tokens of one modality (cores 0-3 ->
modality 0, cores 4-7 -> modality 1) and that modality's full weights.
Dense-eval MoE (all 16 experts computed, sparse gates applied), so no
all-to-all is needed.

v3: expert matmuls run in fp8 (e4m3) with the DoubleRow perf mode (256-row
contraction per instruction at 0.5 cycles/row => 4x the fp32r rate). Experts
are processed in QUADS of 4 (768 h-dims = 3 DoubleRow chunk-pairs). The x
activations / gating / residual / head run in bf16 (exact f32 PSUM
accumulation); gates are quantized to fp8 and broadcast across partitions via
a DRAM roundtrip with partition-step-0 DMAs, laid out per h-chunk so every
gate-multiply is a full [128,512] op. All gb tiles are resident (no reuse
waits). The main loop is a flat software pipeline over (tile, quad): stage1
h-chunks of quad k+1 interleave with stage2 matmul groups of quad k so the PE
never waits on the Act-paced relu chain.

Engine split: PE = matmuls; Act = relu+bias (PSUM->SBUF f32) + gating exp;
Pool = gate-mults (fp8 out, SBUF only); DVE = top-k gating chain + moe drain
(relu+residual, PSUM) + head bias.
"""
import sys

sys.path.insert(0, "/opt/trn_rl_repo")

import numpy as np
import ml_dtypes

import concourse.bass as bass
import concourse.mybir as mybir
import concourse.tile as tile
from concourse import bacc
from concourse.bass_utils import run_bass_kernel_spmd
from concourse.masks import make_identity

# ---- problem sizes (hardcoded per the harness contract) ----
B = 4096           # tokens per modality
D = 768            # model dim
E = 16             # experts
H = 192            # expert hidden
O = 101            # classifier out
KTOP = 12          # top-k experts
NCORES = 8
BC = B // 4        # 1024 tokens per core
DC = D // 128      # 6 d-chunks
NT = 512           # token tile (matmul moving dim)
NTILES = BC // NT  # 2
NQ = 4             # expert quads (4 experts each)
QH = 4 * H         # 768 h-dims per quad = 6 chunks of 128
QCH = QH // 128    # 6 h-chunks per quad
EH128 = E * H // 128  # 24 total h-chunks
F32 = mybir.dt.float32
F32R = mybir.dt.float32r
BF16 = mybir.dt.bfloat16
F8 = mybir.dt.float8e4
DR = mybir.MatmulPerfMode.DoubleRow
NEG_BIG = -1.0e30

_NC_CACHE = {}


def build_nc():
    nc = bacc.Bacc("TRN2", target_bir_lowering=False, debug=False,
                   num_devices=NCORES)

    # ---- DRAM I/O (per-core views; host pre-packs) ----
    xbf_d = nc.dram_tensor("xbf", [128, DC, BC], BF16, kind="ExternalInput").ap()
    x8_d = nc.dram_tensor("x8", [128, DC, BC], F8, kind="ExternalInput").ap()
    w1_d = nc.dram_tensor("w1", [128, DC, E * H], F8, kind="ExternalInput").ap()
    w2_d = nc.dram_tensor("w2", [128, EH128, D], F8, kind="ExternalInput").ap()
    b1_d = nc.dram_tensor("b1", [128, EH128], F32, kind="ExternalInput").ap()
    b2_d = nc.dram_tensor("b2", [E, D], F8, kind="ExternalInput").ap()
    wg_d = nc.dram_tensor("wg", [128, DC, E], BF16, kind="ExternalInput").ap()
    wo_d = nc.dram_tensor("wo", [128, DC, O], BF16, kind="ExternalInput").ap()
    bo_d = nc.dram_tensor("bo", [O, 1], F32, kind="ExternalInput").ap()
    outT = nc.dram_tensor("outT", [O, BC], F32, kind="ExternalOutput").ap()

    with tile.TileContext(nc) as tc:
        with tc.tile_pool(name="const", bufs=1) as cpool, \
             tc.tile_pool(name="gbpool", bufs=8) as gbpool:
            # resident SBUF tensors
            xbf = cpool.tile([128, DC, BC], BF16)      # x (bf16); later holds z
            x8 = cpool.tile([128, DC, BC], F8)         # x (fp8) for W1 matmuls
            w1sb = cpool.tile([128, DC, E * H], F8)
            w2sb = cpool.tile([128, EH128, D], F8)
            b1sb = cpool.tile([128, EH128], F32)
            b2sb = cpool.tile([E, D], F8)
            wgb = cpool.tile([128, DC, E], BF16)
            wob = cpool.tile([128, DC, O], BF16)
            bosb = cpool.tile([O, 1], F32)
            g8 = cpool.tile([E, BC], F8)               # fp8 gates, expert-major
            ident = cpool.tile([128, 128], F32)
            gdram = cpool.tile([E, BC], F8, space="DRAM")

            make_identity(nc, ident[:, :])

            # loads ordered by first use / criticality: the gating chain
            # (xbf -> top-k -> g8 flush -> gb6 broadcast) gates the whole t1
            # pipeline, so xbf pieces go first at high priority. SP queue:
            # gating weights + fp8 tensors + gate broadcasts; Act queue:
            # bf16 x + W2 + Wo (all waitless so the Act engine's relu chain
            # is never blocked).
            with tc.high_priority():
                nc.sync.dma_start(out=wgb[:, :, :], in_=wg_d)
                for p in range(4):
                    ts = slice(256 * p, 256 * (p + 1))
                    nc.scalar.dma_start(out=xbf[:, :, ts], in_=xbf_d[:, :, ts])
            nc.sync.dma_start(out=x8[:, :, 0:NT], in_=x8_d[:, :, 0:NT])
            nc.sync.dma_start(out=w1sb[:, :, 0:QH], in_=w1_d[:, :, 0:QH])
            for q in range(1, NQ):
                hs = slice(QH * q, QH * (q + 1))
                nc.sync.dma_start(out=w1sb[:, :, hs], in_=w1_d[:, :, hs])
            nc.sync.dma_start(out=x8[:, :, NT:], in_=x8_d[:, :, NT:])
            for q in range(NQ):
                nc.scalar.dma_start(out=w2sb[:, 6 * q:6 * (q + 1), :],
                                    in_=w2_d[:, 6 * q:6 * (q + 1), :])
            nc.sync.dma_start(out=b1sb[:, :], in_=b1_d)
            nc.sync.dma_start(out=b2sb[:, :], in_=b2_d)
            nc.scalar.dma_start(out=wob[:, :, :], in_=wo_d)
            nc.sync.dma_start(out=bosb[:, :], in_=bo_d)

            # gb tiles: per (tile, quad) a [128, QCH, NT] fp8 tile where
            # gb[:, j, :] carries the gate of the expert owning h-chunk j,
            # with the two mixed chunks split at partition 64. All 8 resident.
            gb_tiles = {}

            def load_gb6(t, q):
                gb = gbpool.tile([128, QCH, NT], F8, tag="gb6", name="gb6")
                off = 4 * q * BC + NT * t
                # experts (0,1) of the quad own full chunks j=0,2; (2,3) own
                # j=3,5; expert k covers partition-halves 3k..3k+2
                nc.sync.dma_start(
                    out=gb[:, 0:3:2, :],
                    in_=bass.AP(tensor=gdram.tensor, offset=off,
                                ap=[[0, 128], [BC, 2], [1, NT]]))
                nc.sync.dma_start(
                    out=gb[:, 3:6:2, :],
                    in_=bass.AP(tensor=gdram.tensor, offset=off + 2 * BC,
                                ap=[[0, 128], [BC, 2], [1, NT]]))
                for k in range(4):
                    half_j = 1 if k < 2 else 4
                    plo, phi = (0, 64) if k % 2 == 0 else (64, 128)
                    nc.sync.dma_start(
                        out=gb[plo:phi, half_j, :],
                        in_=bass.AP(tensor=gdram.tensor, offset=off + k * BC,
                                    ap=[[0, 64], [1, NT]]))
                gb_tiles[(t, q)] = gb

            # ---------------- gating pass (128-token subtiles) ----------------
            # logits accumulate exactly in f32 PSUM from bf16 inputs; top-12
            # selection + softmax runs in f32 on DVE, gates stored as fp8.
            with tc.tile_pool(name="gps", bufs=2, space="PSUM") as gps, \
                 tc.tile_pool(name="gtp", bufs=2, space="PSUM") as gtp, \
                 tc.tile_pool(name="gsb", bufs=3) as gsb, \
                 tc.high_priority():
                for i in range(BC // 128):
                    ts = slice(128 * i, 128 * (i + 1))
                    lg = gps.tile([128, E], F32, tag="lg")
                    for c in range(DC):
                        nc.tensor.matmul(lg[:, :], xbf[:, c, ts], wgb[:, c, :],
                                         start=(c == 0), stop=(c == DC - 1))
                    # top-8 values, then values 9..16 after masking them out
                    # (all reads straight from PSUM)
                    t8a = gsb.tile([128, 8], F32, tag="t8a")
                    nc.vector.max(t8a[:, :], lg[:, :])
                    l2 = gsb.tile([128, E], F32, tag="l2")
                    nc.vector.match_replace(l2[:, :], t8a[:, :], lg[:, :], NEG_BIG)
                    t8b = gsb.tile([128, 8], F32, tag="t8b")
                    nc.vector.max(t8b[:, :], l2[:, :])
                    # softmax over entries >= 12th-largest (t8b[:,3])
                    e16 = gsb.tile([128, E], F32, tag="e16")
                    nc.scalar.activation(e16[:, :], lg[:, :],
                                         mybir.ActivationFunctionType.Exp)
                    em = gsb.tile([128, E], F32, tag="em")
                    ssum = gsb.tile([128, 1], F32, tag="ssum")
                    nc.vector.scalar_tensor_tensor(
                        out=em[:, :], in0=lg[:, :], scalar=t8b[:, 3:4],
                        in1=e16[:, :], op0=mybir.AluOpType.is_ge,
                        op1=mybir.AluOpType.mult, accum_out=ssum[:, :])
                    rinv = gsb.tile([128, 1], F32, tag="rinv")
                    nc.vector.reciprocal(rinv[:, :], ssum[:, :])
                    g = gsb.tile([128, E], F32, tag="g")
                    nc.vector.tensor_scalar_mul(g[:, :], em[:, :], rinv[:, :])
                    # transpose to expert-major, quantize to fp8
                    gt_ps = gtp.tile([E, 128], F32, tag="gt")
                    nc.tensor.transpose(gt_ps[:, :], g[:, :], ident[:, :])
                    nc.vector.tensor_copy(g8[:, ts], gt_ps[:, :])
                    if 128 * (i + 1) % NT == 0:
                        # flush this token-half of the gates and start the
                        # partition-broadcast reads for its quads
                        t = (128 * (i + 1)) // NT - 1
                        fs = slice(NT * t, NT * (t + 1))
                        nc.sync.dma_start(out=gdram[:, fs], in_=g8[:, fs])
                        for q in range(NQ):
                            load_gb6(t, q)

            # ---------------- main loop: flat (tile, quad) pipeline ----------
            with tc.tile_pool(name="moeps", bufs=DC, space="PSUM") as moeps, \
                 tc.tile_pool(name="hps", bufs=2, space="PSUM") as hps, \
                 tc.tile_pool(name="hsb", bufs=4) as hsb, \
                 tc.tile_pool(name="hgpool", bufs=3) as hgpool, \
                 tc.tile_pool(name="opool", bufs=2) as opool:

                moes = {}      # t -> list of 6 moe PSUM tiles
                hgs = {}       # (t, q) -> hg tile
                out_ps = {}    # t -> head PSUM tile

                def s1_chunk(t, q, j):
                    """One h-chunk: 3 DR matmuls + relu(Act) + gate-mult(Pool)."""
                    ts = slice(NT * t, NT * (t + 1))
                    m = QCH * q + j
                    hcol = 128 * m
                    hps_t = hps.tile([128, NT], F32, tag="h", name="h")
                    for c2 in range(DC // 2):
                        nc.tensor.matmul(
                            hps_t[:, :],
                            w1sb[:, 2 * c2:2 * c2 + 2, hcol:hcol + 128],
                            x8[:, 2 * c2:2 * c2 + 2, ts],
                            start=(c2 == 0), stop=(c2 == DC // 2 - 1),
                            perf_mode=DR)
                    hf = hsb.tile([128, NT], F32, tag="hf", name="hf")
                    nc.scalar.activation(hf[:, :], hps_t[:, :],
                                         mybir.ActivationFunctionType.Relu,
                                         bias=b1sb[:, m:m + 1])
                    nc.gpsimd.tensor_tensor(
                        out=hgs[(t, q)][:, j, :], in0=hf[:, :],
                        in1=gb_tiles[(t, q)][:, j, :], op=mybir.AluOpType.mult)

                def s1_alloc(t, q):
                    hgs[(t, q)] = hgpool.tile([128, QCH, NT], F8, tag="hg",
                                              name="hg")

                def s2_mm(t, q, j2, c):
                    m2 = (QCH // 2) * q + j2
                    nc.tensor.matmul(
                        moes[t][c][:, :],
                        w2sb[:, 2 * m2:2 * m2 + 2, 128 * c:128 * (c + 1)],
                        hgs[(t, q)][:, 2 * j2:2 * j2 + 2, :],
                        start=(q == 0 and j2 == 0), stop=False, perf_mode=DR)

                def finish_chunk(t, c):
                    # z = relu(moe) + x, bf16 in place of x. Alternate the
                    # engine: DVE does it in one scalar_tensor_tensor; for odd
                    # chunks split relu(Act, PSUM->SBUF) + add(Pool, SBUF) so
                    # the close phase isn't paced by a single engine.
                    ts = slice(NT * t, NT * (t + 1))
                    if c % 2 == 0:
                        nc.vector.scalar_tensor_tensor(
                            out=xbf[:, c, ts], in0=moes[t][c][:, :], scalar=0.0,
                            in1=xbf[:, c, ts], op0=mybir.AluOpType.max,
                            op1=mybir.AluOpType.add)
                    else:
                        mt = hsb.tile([128, NT], F32, tag="hf", name="mtmp")
                        nc.scalar.activation(mt[:, :], moes[t][c][:, :],
                                             mybir.ActivationFunctionType.Relu)
                        nc.gpsimd.tensor_tensor(
                            out=xbf[:, c, ts], in0=mt[:, :],
                            in1=xbf[:, c, ts], op=mybir.AluOpType.add)

                def head_chunk(t, c):
                    ts = slice(NT * t, NT * (t + 1))
                    if t not in out_ps:
                        out_ps[t] = hps.tile([O, NT], F32, tag="h",
                                             name="out_ps")
                    nc.tensor.matmul(out_ps[t][:, :], wob[:, c, :],
                                     xbf[:, c, ts],
                                     start=(c == 0), stop=(c == DC - 1))

                def s2_groups(t, q, close):
                    """Yield stage2 work as 6 groups (to interleave with the
                    next quad's 6 s1 chunks)."""
                    ts = slice(NT * t, NT * (t + 1))
                    if not close:
                        # j2-major so the last hg chunk-pair is needed late
                        for j2 in range(QCH // 2):
                            for ch in range(2):
                                def grp(j2=j2, ch=ch):
                                    for c in range(3 * ch, 3 * ch + 3):
                                        s2_mm(t, q, j2, c)
                                yield grp
                        return
                    # close: j2=0,1 accumulation first (j2-major), then a
                    # c-major pass with the final pair + b2 + drain + head
                    def grp_a():
                        for c in range(DC):
                            s2_mm(t, q, 0, c)
                    yield grp_a

                    def grp_b():
                        for c in range(DC):
                            s2_mm(t, q, 1, c)
                    yield grp_b

                    for ch in range(4):
                        def grp_c(ch=ch):
                            cs = [(0, 1), (2,), (3, 4), (5,)][ch]
                            for c in cs:
                                s2_mm(t, q, 2, c)
                                nc.tensor.matmul(
                                    moes[t][c][:, :],
                                    b2sb[:, 128 * c:128 * (c + 1)],
                                    g8[:, ts], start=False, stop=True)
                                finish_chunk(t, c)
                                if c >= 2:
                                    head_chunk(t, c - 2)
                            if ch == 3:
                                head_chunk(t, DC - 2)
                                head_chunk(t, DC - 1)
                        yield grp_c

                def emit_out(t):
                    ts = slice(NT * t, NT * (t + 1))
                    osb = opool.tile([O, NT], F32, tag="osb")
                    nc.vector.tensor_scalar_add(osb[:, :], out_ps[t][:, :],
                                                bosb[:, 0:1])
                    nc.sync.dma_start(out=outT[:, ts], in_=osb[:, :])

                seq = [(t, q) for t in range(NTILES) for q in range(NQ)]
                for t in range(NTILES):
                    moes[t] = [moeps.tile([128, NT], F32, tag="moe", name="moe")
                               for _ in range(DC)]
                # prologue: first quad's stage1, un-interleaved
                s1_alloc(0, 0)
                for j in range(QCH):
                    s1_chunk(0, 0, j)
                for idx, (t, q) in enumerate(seq):
                    nxt = seq[idx + 1] if idx + 1 < len(seq) else None
                    groups = list(s2_groups(t, q, close=(q == NQ - 1)))
                    if nxt is not None:
                        s1_alloc(*nxt)
                        for j in range(QCH):
                            s1_chunk(nxt[0], nxt[1], j)
                            groups[j]()
                    else:
                        for grp in groups:
                            grp()
                    if q == NQ - 1:
                        emit_out(t)

    nc.compile()
    return nc


def _pack_core_inputs(x, Wg, W1, b1, W2, b2, Wo, bo, c4):
    """Per-core input dict for one modality's weights + 1024-token slice."""
    f = np.float32
    bf = ml_dtypes.bfloat16
    f8 = ml_dtypes.float8_e4m3
    tok = slice(BC * c4, BC * (c4 + 1))
    xT = np.asarray(x[tok], f).T                       # [D, BC]
    xr = np.ascontiguousarray(
        xT.reshape(DC, 128, BC).transpose(1, 0, 2))    # [128, DC, BC]
    w1f = np.asarray(W1, f).transpose(1, 0, 2).reshape(D, E * H)
    w2f = np.asarray(W2, f).reshape(E * H, D)
    return {
        "xbf": xr.astype(bf),
        "x8": xr.astype(f8),
        "w1": np.ascontiguousarray(
            w1f.reshape(DC, 128, E * H).transpose(1, 0, 2)).astype(f8),
        "w2": np.ascontiguousarray(
            w2f.reshape(EH128, 128, D).transpose(1, 0, 2)).astype(f8),
        "b1": np.ascontiguousarray(
            np.asarray(b1, f).reshape(EH128, 128).T),
        "b2": np.asarray(b2, f).astype(f8),
        "wg": np.ascontiguousarray(
            np.asarray(Wg, f).reshape(DC, 128, E).transpose(1, 0, 2)).astype(bf),
        "wo": np.ascontiguousarray(
            np.asarray(Wo, f).reshape(DC, 128, O).transpose(1, 0, 2)).astype(bf),
        "bo": np.ascontiguousarray(np.asarray(bo, f).reshape(O, 1)),
    }


def run_on_hw(inputs, trace=False, **kw):
    if "nc" not in _NC_CACHE:
        _NC_CACHE["nc"] = build_nc()
    nc = _NC_CACHE["nc"]
    in_maps = []
    for core in range(NCORES):
        i, c4 = divmod(core, 4)
        x = inputs["x0"] if i == 0 else inputs["x1"]
        in_maps.append(_pack_core_inputs(
            x, inputs["Wg"][i], inputs["W1"][i], inputs["b1"][i],
            inputs["W2"][i], inputs["b2"][i], inputs["Wo"][i], inputs["bo"][i], c4))
    res = run_bass_kernel_spmd(nc, in_maps, core_ids=list(range(NCORES)),
                               trace=trace, **kw)
    outs = []
    for i in range(2):
        outs.append(np.concatenate(
            [res.results[4 * i + c]["outT"].T for c in range(4)], axis=0))
    return (outs[0], outs[1]), res


def kernel(**inputs):
    (o0, o1), _ = run_on_hw(inputs)
    return (o0, o1)
